# revision 33
# baseline (speedup 1.0000x reference)
"""Trainium2 Bass kernel for BipartiteGNNConvFactorToVariable (v3).

  out = variables + relu(concat([variables, aggr]) @ W_comb + b_comb)
  aggr = segment_sum(relu(concat([x_i, x_j, 0]) @ W_msg + b_msg), v_to_f)
  x_i = variables[v_to_f], x_j = factors[f_to_v]

Distribution (8 cores, zero collectives), "identity slotting": variables
are sorted by degree and grouped into 784 blocks of 128; sorted stripe j
(blocks 8j..8j+7) is dealt one block per core, so all cores share the
same per-block tile count T_j = max degree in the stripe (1.5% slot
padding).  Block j owns T_j edge tiles of 128 slots; the edge targeting
variable (block j, column c) with within-variable rank r sits at tile r,
partition... column c.  Crucially the edge slot column EQUALS the target
variable column, so both the per-edge gather of p = V@W1 and the
segment-sum reduce to matmuls against the IDENTITY matrix - no per-edge
index operands at all:

  pm[e_tile] = I.T @ p  (+ xjT.T @ W2, mixed fp8 x bf16)   [PE]
  m = relu(pm)          (PSUM-bank batches, Vector/Scalar alternating)
  aggrT[d, v] += m.T @ I  per tile                         [PE]

Pad slots (columns with rank >= deg) contribute relu(p_v) each; an exact
correction tile m_corr = -(T_j - deg_v) * relu(p_v) is accumulated into
the same PSUM group.  The comb MLP runs transposed with N=512 matmuls:
phT = Wc1.T@V.T + Wc2.T@aggrT, outT = max(phT,0) + V.T fused on Vector,
stored bf16 and unpermuted on the host.

Only x_j is quantized (fp8, rel err ~6e-3 vs the 2e-2 gate); V, p and
all weights stay bf16.
"""

import numpy as np
import ml_dtypes

import concourse.bass as bass
import concourse.tile as tile
from concourse import mybir
from concourse.bass_utils import run_bass_kernel_spmd

BF16 = ml_dtypes.bfloat16
FP8 = ml_dtypes.float8_e4m3

NV, NF, E, D = 100000, 50000, 1000000, 128
NC = 8
NBLK_CORE = 98              # blocks per core (= stripes)
NBLK = NC * NBLK_CORE       # 784
NVC = NBLK_CORE * 128       # 12544 variable slots per core
GROUP = 4                   # blocks per staging group


def pack_v3(v_to_f):
    """Degree-sorted identity slotting.

    Returns (core_of, blkj_of, col_of, rank, T_j[98], vid_of[NC,98,128]).
    Variable order[(j*8+c)*128 + k] -> core c, block j, column k.
    """
    deg = np.bincount(v_to_f, minlength=NV).astype(np.int64)
    order = np.argsort(-deg, kind="stable")
    nslot_v = NBLK * 128
    vid_seq = np.concatenate([order, np.full(nslot_v - NV, -1, np.int64)])
    degs = np.concatenate([deg[order], np.zeros(nslot_v - NV, np.int64)])

    T_j = np.maximum(1, degs.reshape(NBLK_CORE, 8 * 128).max(axis=1))
    T_j = T_j.astype(np.int64)

    seq = np.arange(nslot_v)
    blkj = seq // (8 * 128)
    core = (seq // 128) % 8
    col = seq % 128

    core_of = np.empty(NV, np.int32)
    blkj_of = np.empty(NV, np.int32)
    col_of = np.empty(NV, np.int32)
    real = vid_seq >= 0
    core_of[vid_seq[real]] = core[real]
    blkj_of[vid_seq[real]] = blkj[real]
    col_of[vid_seq[real]] = col[real]

    # rank of each edge within its variable's edge list
    ev = v_to_f.astype(np.int64)
    eorder = np.argsort(ev, kind="stable")
    counts = np.bincount(ev, minlength=NV)
    starts = np.concatenate([[0], np.cumsum(counts)[:-1]])
    rank = np.empty(E, np.int64)
    rank[eorder] = np.arange(E) - starts[ev[eorder]]

    vid_of = np.full((NC, NBLK_CORE, 128), -1, np.int64)
    vid_of[core[real], blkj[real], col[real]] = vid_seq[real]
    return core_of, blkj_of, col_of, rank, T_j, vid_of


def build_host_data(variables, factors, v_to_f, f_to_v,
                    W_msg, b_msg, W_comb, b_comb):
    core_of, blkj_of, col_of, rank, T_j, vid_of = pack_v3(v_to_f)
    off_j = np.concatenate([[0], np.cumsum(T_j)[:-1]])   # tile offset per block
    ntiles = int(T_j.sum())
    nslots = ntiles * 128

    ev = v_to_f
    e_core = core_of[ev]
    e_slot = (off_j[blkj_of[ev]] + rank) * 128 + col_of[ev]

    factors_f8 = factors.astype(FP8)
    deg = np.bincount(v_to_f, minlength=NV).astype(np.int64)

    in_maps = []
    for c in range(NC):
        sel = e_core == c
        slots = e_slot[sel]
        xj8 = np.zeros((D, nslots), FP8)
        xj8[:, slots] = factors_f8[f_to_v[sel]].T

        vids = vid_of[c].reshape(-1)
        mask = vids >= 0
        vperm = np.zeros((NVC, D), np.float32)
        vperm[mask] = variables[vids[mask]]

        # -(T_j - deg_v) at [col, block]
        dv = np.zeros((NBLK_CORE, 128), np.float32)
        dv[:, :] = -T_j[:, None]
        vm = vids.reshape(NBLK_CORE, 128)
        rmask = vm >= 0
        dv[rmask] = -(T_j[:, None] - deg[np.maximum(vm, 0)])[rmask]
        # dummy columns (vid -1): deg treated as 0 but p = 0 so term is 0;
        # keep -T_j there, harmless since relu(p)=relu(0)=0.
        negpad = np.ascontiguousarray(dv.T)  # [col, block]

        in_maps.append(dict(
            xj8=xj8,
            vT=np.ascontiguousarray(vperm.T).astype(BF16),
            negpad=negpad,
            ident=np.eye(128, dtype=np.float32).astype(BF16),
            w1=np.ascontiguousarray(W_msg[0:D]).astype(BF16),
            w2=np.ascontiguousarray(W_msg[D:2 * D]).astype(BF16),
            wc1=np.ascontiguousarray(W_comb[0:D]).astype(BF16),
            wc2=np.ascontiguousarray(W_comb[D:2 * D]).astype(BF16),
        ))

    has_msg_bias = bool(np.any(b_msg != 0))
    has_comb_bias = bool(np.any(b_comb != 0))
    if has_msg_bias:
        for m in in_maps:
            m["bmsg_bf"] = b_msg.reshape(1, D).astype(BF16)
    if has_comb_bias:
        for m in in_maps:
            m["bcomb_bf"] = b_comb.reshape(1, D).astype(BF16)
    if has_msg_bias or has_comb_bias:
        for m in in_maps:
            m["ones_bf"] = np.ones((1, 512), BF16)
    return in_maps, vid_of, [int(t) for t in T_j], has_msg_bias, has_comb_bias


def split_multi_waits(nc, max_waits=1):
    """This walrus rejects >1 sync-wait command on an instruction; move the
    extras onto injected NoOps just before it (same engine, program order)."""
    for fn in nc.m.functions:
        for bb in fn.blocks:
            new_insts = []
            for inst in bb.instructions:
                si = inst.sync_info
                if (si is not None and si.on_wait
                        and len(si.on_wait) > max_waits):
                    waits = list(si.on_wait)
                    move, keep = waits[:-max_waits], waits[-max_waits:]
                    for j, w in enumerate(move):
                        nop = mybir.InstNoOp(
                            name=f"{inst.name}-wsplit{j}",
                            sync_info=mybir.SyncInfo(on_wait=[w],
                                                     on_update=[]),
                            bass_nofuse=True,
                            engine=inst.engine,
                        )
                        nc.register_instruction(nop)
                        new_insts.append(nop)
                    si.on_wait = keep
                new_insts.append(inst)
            bb.instructions[:] = new_insts
    return nc


def build_nc(T_j, has_msg_bias, has_comb_bias, repeat=1):
    ntiles = sum(T_j)
    NSLOT = ntiles * 128
    NGROUPS = (NBLK_CORE + GROUP - 1) // GROUP
    off_j = np.concatenate([[0], np.cumsum(T_j)[:-1]]).astype(int)
    max_grp_tiles = max(
        sum(T_j[g * GROUP:(g + 1) * GROUP]) for g in range(NGROUPS))

    f32, bf, f8 = mybir.dt.float32, mybir.dt.bfloat16, mybir.dt.float8e4
    relu_t = mybir.ActivationFunctionType.Relu
    amax, amult = mybir.AluOpType.max, mybir.AluOpType.mult
    nc = bass.Bass("TRN2", target_bir_lowering=False, debug=False,
                   num_devices=NC)

    xj8_d = nc.dram_tensor("xj8", [D, NSLOT], f8, kind="ExternalInput").ap()
    vT_d = nc.dram_tensor("vT", [D, NVC], bf, kind="ExternalInput").ap()
    np_d = nc.dram_tensor("negpad", [128, NBLK_CORE], f32,
                          kind="ExternalInput").ap()
    id_d = nc.dram_tensor("ident", [128, 128], bf, kind="ExternalInput").ap()
    w1_d = nc.dram_tensor("w1", [D, D], bf, kind="ExternalInput").ap()
    w2_d = nc.dram_tensor("w2", [D, D], bf, kind="ExternalInput").ap()
    wc1_d = nc.dram_tensor("wc1", [D, D], bf, kind="ExternalInput").ap()
    wc2_d = nc.dram_tensor("wc2", [D, D], bf, kind="ExternalInput").ap()
    if has_msg_bias:
        bmsg_d = nc.dram_tensor("bmsg_bf", [1, D], bf,
                                kind="ExternalInput").ap()
    if has_comb_bias:
        bcomb_d = nc.dram_tensor("bcomb_bf", [1, D], bf,
                                 kind="ExternalInput").ap()
    if has_msg_bias or has_comb_bias:
        ones_d = nc.dram_tensor("ones_bf", [1, 512], bf,
                                kind="ExternalInput").ap()
    outT = nc.dram_tensor("outT", [D, NVC], bf, kind="ExternalOutput").ap()

    with tile.TileContext(nc) as tc:
        with (tc.tile_pool(name="const", bufs=1) as constp,
              tc.tile_pool(name="stage", bufs=3) as stagep,
              tc.tile_pool(name="work", bufs=3) as workp,
              tc.tile_pool(name="psum_m", bufs=4, space="PSUM") as psmp,
              tc.tile_pool(name="psum_a", bufs=2, space="PSUM") as psap,
              tc.tile_pool(name="psum_p", bufs=1, space="PSUM") as pspp,
              tc.tile_pool(name="psum_b", bufs=1, space="PSUM") as psbp):

            # Head ordering: the tile framework gates consumers on all
            # earlier-issued DMAs, so issue only what the p-pipeline needs
            # (vT, w1, negpad), then emit the group-0 p compute, then the
            # remaining consts and the g0 xj stream.
            vT_st0 = stagep.tile([128, GROUP * 128], bf, tag="vT")
            nc.sync.dma_start(vT_st0[:, :min(GROUP, NBLK_CORE) * 128],
                              vT_d[:, :min(GROUP, NBLK_CORE) * 128])
            w1_s = constp.tile([D, D], bf)
            nc.sync.dma_start(w1_s[:], w1_d[:])
            np_s = constp.tile([128, NBLK_CORE], f32)
            nc.sync.dma_start(np_s[:], np_d[:])
            id_s = constp.tile([128, 128], bf)
            w2_s = constp.tile([D, D], bf)
            wc1_s = constp.tile([D, D], bf)
            wc2_s = constp.tile([D, D], bf)
            xj_st0 = stagep.tile([128, max_grp_tiles * 128], f8, tag="xj")

            def _late_head_dmas():
                nc.sync.dma_start(id_s[:], id_d[:])
                nc.sync.dma_start(w2_s[:], w2_d[:])
                _nb0 = min(GROUP, NBLK_CORE)
                _o = 0
                for _b in range(_nb0):
                    _w = T_j[_b] * 128
                    if _b <= 1:
                        _h = 4 * 128
                        nc.sync.dma_start(xj_st0[:, _o:_o + _h],
                                          xj8_d[:, _o:_o + _h])
                        nc.sync.dma_start(xj_st0[:, _o + _h:_o + _w],
                                          xj8_d[:, _o + _h:_o + _w])
                    else:
                        nc.sync.dma_start(xj_st0[:, _o:_o + _w],
                                          xj8_d[:, _o:_o + _w])
                    _o += _w
                nc.sync.dma_start(wc1_s[:], wc1_d[:])
                nc.sync.dma_start(wc2_s[:], wc2_d[:])
            if has_msg_bias:
                bmsg_s = constp.tile([1, D], bf)
                nc.sync.dma_start(bmsg_s[:], bmsg_d[:])
            if has_comb_bias:
                bcomb_s = constp.tile([1, D], bf)
                nc.sync.dma_start(bcomb_s[:], bcomb_d[:])
            if has_msg_bias or has_comb_bias:
                ones_s = constp.tile([1, 512], bf)
                nc.sync.dma_start(ones_s[:], ones_d[:])

            for _rep in range(repeat):
                eng_flip = 0
                state = {}

                def stageA(g, pre=None):
                    nb = min(GROUP, NBLK_CORE - g * GROUP)
                    Tg = T_j[g * GROUP:g * GROUP + nb]
                    nch = sum(Tg)
                    nsl = nch * 128
                    s0 = off_j[g * GROUP] * 128
                    nvw = nb * 128

                    if pre is not None:
                        xj_st, vT_st = pre
                    else:
                        xj_st = stagep.tile([128, max_grp_tiles * 128],
                                            f8, tag="xj")
                        nc.sync.dma_start(xj_st[:, :nsl],
                                          xj8_d[:, s0:s0 + nsl])
                        vT_st = stagep.tile([128, GROUP * 128], bf,
                                            tag="vT")
                        nc.sync.dma_start(
                            vT_st[:, :nvw],
                            vT_d[:, g * GROUP * 128:g * GROUP * 128
                                 + nvw])

                    pp = pspp.tile([128, 512], f32, tag="pp")
                    for b in range(nb):
                        nc.tensor.matmul(
                            pp[:, b * 128:(b + 1) * 128],
                            vT_st[:, b * 128:(b + 1) * 128], w1_s[:],
                            start=(b == 0), stop=not has_msg_bias,
                            skip_group_check=True)
                        if has_msg_bias:
                            nc.tensor.matmul(
                                pp[:, b * 128:(b + 1) * 128],
                                ones_s[:, :128], bmsg_s[:],
                                start=False, stop=True,
                                skip_group_check=True)
                    p_s = workp.tile([128, 512], bf, tag="ps")
                    h = (nvw // 256) * 128
                    if h:
                        nc.scalar.copy(p_s[:, :h], pp[:, :h])
                        nc.vector.tensor_copy(p_s[:, h:nvw], pp[:, h:nvw])
                    else:
                        nc.scalar.copy(p_s[:, :nvw], pp[:, :nvw])

                    m_corr = workp.tile([128, 512], bf, tag="mc")
                    for b in range(nb):
                        j = g * GROUP + b
                        nc.vector.tensor_scalar(
                            m_corr[:, b * 128:(b + 1) * 128],
                            p_s[:, b * 128:(b + 1) * 128],
                            0.0, np_s[:, j:j + 1], op0=amax, op1=amult)
                    if pre is not None:
                        _late_head_dmas()
                    state[g] = dict(nb=nb, Tg=Tg, nch=nch, nvw=nvw,
                                    s0=s0, xj_st=xj_st, vT_st=vT_st,
                                    p_s=p_s, m_corr=m_corr)

                def stageB(g, mid_hook=None, early_hook=None):
                    nonlocal eng_flip
                    st = state[g]
                    nb, Tg, nch = st["nb"], st["Tg"], st["nch"]
                    nvw = st["nvw"]
                    xj_st, p_s, m_corr = (st["xj_st"], st["p_s"],
                                          st["m_corr"])
                    tlist = []
                    for b in range(nb):
                        for t in range(Tg[b]):
                            tlist.append((b, t))
                    nbank = (nch + 3) // 4
                    pa = psap.tile([128, 512], f32, tag="pa")
                    mss = [None] * nbank

                    def emit_mm12(jb):
                        nonlocal eng_flip
                        w = min(4, nch - jb * 4)
                        pm = psmp.tile([128, 512], f32, tag="pm")
                        jj = 0
                        while jj < w:
                            b = tlist[jb * 4 + jj][0]
                            r = jj
                            while r < w and tlist[jb * 4 + r][0] == b:
                                r += 1
                            run = r - jj
                            rhs = p_s[:, b * 128:(b + 1) * 128]
                            if run > 1:
                                rhs = rhs.unsqueeze(1).broadcast_to(
                                    [128, run, 128])
                            nc.tensor.matmul(
                                pm[:, jj * 128:(jj + run) * 128],
                                id_s[:], rhs, start=(jj == 0),
                                stop=False, skip_group_check=True)
                            jj = r
                        for jj in range(w):
                            cc = jb * 4 + jj
                            nc.tensor.matmul(
                                pm[:, jj * 128:(jj + 1) * 128],
                                xj_st[:, cc * 128:cc * 128 + 128],
                                w2_s[:],
                                start=False, stop=True,
                                skip_group_check=True)
                        m_s = workp.tile([128, 512], bf, tag="m", bufs=4)
                        mss[jb] = m_s
                        if eng_flip % 2 == 0:
                            nc.vector.tensor_scalar_max(
                                m_s[:, :w * 128], pm[:, :w * 128], 0.0)
                        else:
                            nc.scalar.activation(m_s[:, :w * 128],
                                                 pm[:, :w * 128], relu_t)
                        eng_flip += 1

                    def emit_mm3(jb):
                        w = min(4, nch - jb * 4)
                        m_s = mss[jb]
                        for jj in range(w):
                            b, t = tlist[jb * 4 + jj]
                            if t == 0:
                                nc.tensor.matmul(
                                    pa[:, b * 128:(b + 1) * 128],
                                    m_corr[:, b * 128:(b + 1) * 128],
                                    id_s[:], start=(b == 0), stop=False,
                                    skip_group_check=True)
                            nc.tensor.matmul(
                                pa[:, b * 128:(b + 1) * 128],
                                m_s[:, jj * 128:(jj + 1) * 128],
                                id_s[:],
                                start=False, stop=(t == Tg[b] - 1),
                                skip_group_check=True)

                    LAG = 2
                    for jb in range(nbank):
                        emit_mm12(jb)
                        if jb >= LAG:
                            emit_mm3(jb - LAG)
                        if jb == max(1, nbank // 3) and mid_hook is not None:
                            mid_hook()
                        if jb == LAG - 1 and early_hook is not None:
                            early_hook()
                    for jb in range(max(0, nbank - LAG), nbank):
                        emit_mm3(jb)

                    ag_s = workp.tile([128, 512], bf, tag="ag")
                    h = (nvw // 256) * 128
                    if h:
                        nc.scalar.copy(ag_s[:, :h], pa[:, :h])
                        nc.vector.tensor_copy(ag_s[:, h:nvw], pa[:, h:nvw])
                    else:
                        nc.scalar.copy(ag_s[:, :nvw], pa[:, :nvw])
                    st["ag_s"] = ag_s

                def stageC(g):
                    st = state.pop(g)
                    nvw, vT_st, ag_s = st["nvw"], st["vT_st"], st["ag_s"]
                    ph = psbp.tile([128, 512], f32, tag="ph")
                    nc.tensor.matmul(ph[:, :nvw], wc1_s[:],
                                     vT_st[:, :nvw],
                                     start=True, stop=False)
                    nc.tensor.matmul(ph[:, :nvw], wc2_s[:],
                                     ag_s[:, :nvw],
                                     start=False, stop=not has_comb_bias,
                                     skip_group_check=True)
                    if has_comb_bias:
                        nc.tensor.matmul(ph[:, :nvw], bcomb_s[:],
                                         ones_s[:, :nvw],
                                         start=False, stop=True,
                                         skip_group_check=True)
                    o_s = workp.tile([128, 512], bf, tag="o")
                    nc.vector.scalar_tensor_tensor(
                        o_s[:, :nvw], ph[:, :nvw], 0.0, vT_st[:, :nvw],
                        op0=amax, op1=mybir.AluOpType.add)
                    nc.sync.dma_start(
                        outT[:, g * GROUP * 128:g * GROUP * 128 + nvw],
                        o_s[:, :nvw])

                stageA(0, pre=(xj_st0, vT_st0))
                for g in range(NGROUPS):
                    hookA = ((lambda gg=g: stageA(gg + 1))
                             if g + 1 < NGROUPS else None)
                    hookC = ((lambda gg=g: stageC(gg - 1))
                             if g >= 1 else None)
                    stageB(g, mid_hook=hookA, early_hook=hookC)
                stageC(NGROUPS - 1)

    split_multi_waits(nc)
    return nc


_RUN_KW = {}   # test harness can inject run_bass_kernel_spmd kwargs
_REPEAT = 1    # test harness can ask for a repeated body (timing)


def kernel(variables, factors, v_to_f, f_to_v, edge_attr,
           W_msg, b_msg, W_comb, b_comb):
    variables = np.asarray(variables, np.float32)
    factors = np.asarray(factors, np.float32)
    v_to_f = np.asarray(v_to_f, np.int32)
    f_to_v = np.asarray(f_to_v, np.int32)
    W_msg = np.asarray(W_msg, np.float32)
    b_msg = np.asarray(b_msg, np.float32)
    W_comb = np.asarray(W_comb, np.float32)
    b_comb = np.asarray(b_comb, np.float32)

    in_maps, vid_of, T_j, has_mb, has_cb = build_host_data(
        variables, factors, v_to_f, f_to_v, W_msg, b_msg, W_comb, b_comb)

    nc = build_nc(T_j, has_mb, has_cb, repeat=_REPEAT)
    res = run_bass_kernel_spmd(nc, in_maps, list(range(NC)), **_RUN_KW)

    out_full = np.zeros((NV, D), np.float32)
    for c in range(NC):
        vids = vid_of[c].reshape(-1)
        mask = vids >= 0
        outc = res.results[c]["outT"].T.astype(np.float32)
        out_full[vids[mask]] = outc[mask]
    kernel.last_results = res
    return out_full


# revision 34
# speedup vs baseline: 1.0110x; 1.0110x over previous
"""Trainium2 Bass kernel for BipartiteGNNConvFactorToVariable (v3).

  out = variables + relu(concat([variables, aggr]) @ W_comb + b_comb)
  aggr = segment_sum(relu(concat([x_i, x_j, 0]) @ W_msg + b_msg), v_to_f)
  x_i = variables[v_to_f], x_j = factors[f_to_v]

Distribution (8 cores, zero collectives), "identity slotting": variables
are sorted by degree and grouped into 784 blocks of 128; sorted stripe j
(blocks 8j..8j+7) is dealt one block per core, so all cores share the
same per-block tile count T_j = max degree in the stripe (1.5% slot
padding).  Block j owns T_j edge tiles of 128 slots; the edge targeting
variable (block j, column c) with within-variable rank r sits at tile r,
partition... column c.  Crucially the edge slot column EQUALS the target
variable column, so both the per-edge gather of p = V@W1 and the
segment-sum reduce to matmuls against the IDENTITY matrix - no per-edge
index operands at all:

  pm[e_tile] = I.T @ p  (+ xjT.T @ W2, mixed fp8 x bf16)   [PE]
  m = relu(pm)          (PSUM-bank batches, Vector/Scalar alternating)
  aggrT[d, v] += m.T @ I  per tile                         [PE]

Pad slots (columns with rank >= deg) contribute relu(p_v) each; an exact
correction tile m_corr = -(T_j - deg_v) * relu(p_v) is accumulated into
the same PSUM group.  The comb MLP runs transposed with N=512 matmuls:
phT = Wc1.T@V.T + Wc2.T@aggrT, outT = max(phT,0) + V.T fused on Vector,
stored bf16 and unpermuted on the host.

Only x_j is quantized (fp8, rel err ~6e-3 vs the 2e-2 gate); V, p and
all weights stay bf16.
"""

import numpy as np
import ml_dtypes

import concourse.bass as bass
import concourse.tile as tile
from concourse import mybir
from concourse.bass_utils import run_bass_kernel_spmd

BF16 = ml_dtypes.bfloat16
FP8 = ml_dtypes.float8_e4m3

NV, NF, E, D = 100000, 50000, 1000000, 128
NC = 8
NBLK_CORE = 98              # blocks per core (= stripes)
NBLK = NC * NBLK_CORE       # 784
NVC = NBLK_CORE * 128       # 12544 variable slots per core
GROUP = 4                   # blocks per staging group


def pack_v3(v_to_f):
    """Degree-sorted identity slotting.

    Returns (core_of, blkj_of, col_of, rank, T_j[98], vid_of[NC,98,128]).
    Variable order[(j*8+c)*128 + k] -> core c, block j, column k.
    """
    deg = np.bincount(v_to_f, minlength=NV).astype(np.int64)
    order = np.argsort(-deg, kind="stable")
    nslot_v = NBLK * 128
    vid_seq = np.concatenate([order, np.full(nslot_v - NV, -1, np.int64)])
    degs = np.concatenate([deg[order], np.zeros(nslot_v - NV, np.int64)])

    T_j = np.maximum(1, degs.reshape(NBLK_CORE, 8 * 128).max(axis=1))
    T_j = T_j.astype(np.int64)

    seq = np.arange(nslot_v)
    blkj = seq // (8 * 128)
    core = (seq // 128) % 8
    col = seq % 128

    core_of = np.empty(NV, np.int32)
    blkj_of = np.empty(NV, np.int32)
    col_of = np.empty(NV, np.int32)
    real = vid_seq >= 0
    core_of[vid_seq[real]] = core[real]
    blkj_of[vid_seq[real]] = blkj[real]
    col_of[vid_seq[real]] = col[real]

    # rank of each edge within its variable's edge list
    ev = v_to_f.astype(np.int64)
    eorder = np.argsort(ev, kind="stable")
    counts = np.bincount(ev, minlength=NV)
    starts = np.concatenate([[0], np.cumsum(counts)[:-1]])
    rank = np.empty(E, np.int64)
    rank[eorder] = np.arange(E) - starts[ev[eorder]]

    vid_of = np.full((NC, NBLK_CORE, 128), -1, np.int64)
    vid_of[core[real], blkj[real], col[real]] = vid_seq[real]
    return core_of, blkj_of, col_of, rank, T_j, vid_of


def build_host_data(variables, factors, v_to_f, f_to_v,
                    W_msg, b_msg, W_comb, b_comb):
    core_of, blkj_of, col_of, rank, T_j, vid_of = pack_v3(v_to_f)
    off_j = np.concatenate([[0], np.cumsum(T_j)[:-1]])   # tile offset per block
    ntiles = int(T_j.sum())
    nslots = ntiles * 128

    ev = v_to_f
    e_core = core_of[ev]
    e_slot = (off_j[blkj_of[ev]] + rank) * 128 + col_of[ev]

    factors_f8 = factors.astype(FP8)
    deg = np.bincount(v_to_f, minlength=NV).astype(np.int64)

    in_maps = []
    for c in range(NC):
        sel = e_core == c
        slots = e_slot[sel]
        xj8 = np.zeros((D, nslots), FP8)
        xj8[:, slots] = factors_f8[f_to_v[sel]].T

        vids = vid_of[c].reshape(-1)
        mask = vids >= 0
        vperm = np.zeros((NVC, D), np.float32)
        vperm[mask] = variables[vids[mask]]

        # -(T_j - deg_v) at [col, block]
        dv = np.zeros((NBLK_CORE, 128), np.float32)
        dv[:, :] = -T_j[:, None]
        vm = vids.reshape(NBLK_CORE, 128)
        rmask = vm >= 0
        dv[rmask] = -(T_j[:, None] - deg[np.maximum(vm, 0)])[rmask]
        # dummy columns (vid -1): deg treated as 0 but p = 0 so term is 0;
        # keep -T_j there, harmless since relu(p)=relu(0)=0.
        negpad = np.ascontiguousarray(dv.T)  # [col, block]

        in_maps.append(dict(
            xj8=xj8,
            vT=np.ascontiguousarray(vperm.T).astype(BF16),
            negpad=negpad,
            ident=np.eye(128, dtype=np.float32).astype(BF16),
            w1=np.ascontiguousarray(W_msg[0:D]).astype(BF16),
            w2=np.ascontiguousarray(W_msg[D:2 * D]).astype(BF16),
            wc1=np.ascontiguousarray(W_comb[0:D]).astype(BF16),
            wc2=np.ascontiguousarray(W_comb[D:2 * D]).astype(BF16),
        ))

    has_msg_bias = bool(np.any(b_msg != 0))
    has_comb_bias = bool(np.any(b_comb != 0))
    if has_msg_bias:
        for m in in_maps:
            m["bmsg_bf"] = b_msg.reshape(1, D).astype(BF16)
    if has_comb_bias:
        for m in in_maps:
            m["bcomb_bf"] = b_comb.reshape(1, D).astype(BF16)
    if has_msg_bias or has_comb_bias:
        for m in in_maps:
            m["ones_bf"] = np.ones((1, 512), BF16)
    return in_maps, vid_of, [int(t) for t in T_j], has_msg_bias, has_comb_bias


def split_multi_waits(nc, max_waits=1):
    """This walrus rejects >1 sync-wait command on an instruction; move the
    extras onto injected NoOps just before it (same engine, program order)."""
    for fn in nc.m.functions:
        for bb in fn.blocks:
            new_insts = []
            for inst in bb.instructions:
                si = inst.sync_info
                if (si is not None and si.on_wait
                        and len(si.on_wait) > max_waits):
                    waits = list(si.on_wait)
                    move, keep = waits[:-max_waits], waits[-max_waits:]
                    for j, w in enumerate(move):
                        nop = mybir.InstNoOp(
                            name=f"{inst.name}-wsplit{j}",
                            sync_info=mybir.SyncInfo(on_wait=[w],
                                                     on_update=[]),
                            bass_nofuse=True,
                            engine=inst.engine,
                        )
                        nc.register_instruction(nop)
                        new_insts.append(nop)
                    si.on_wait = keep
                new_insts.append(inst)
            bb.instructions[:] = new_insts
    return nc


def build_nc(T_j, has_msg_bias, has_comb_bias, repeat=1):
    ntiles = sum(T_j)
    NSLOT = ntiles * 128
    NGROUPS = (NBLK_CORE + GROUP - 1) // GROUP
    off_j = np.concatenate([[0], np.cumsum(T_j)[:-1]]).astype(int)
    max_grp_tiles = max(
        sum(T_j[g * GROUP:(g + 1) * GROUP]) for g in range(NGROUPS))

    f32, bf, f8 = mybir.dt.float32, mybir.dt.bfloat16, mybir.dt.float8e4
    relu_t = mybir.ActivationFunctionType.Relu
    amax, amult = mybir.AluOpType.max, mybir.AluOpType.mult
    nc = bass.Bass("TRN2", target_bir_lowering=False, debug=False,
                   num_devices=NC)

    xj8_d = nc.dram_tensor("xj8", [D, NSLOT], f8, kind="ExternalInput").ap()
    vT_d = nc.dram_tensor("vT", [D, NVC], bf, kind="ExternalInput").ap()
    np_d = nc.dram_tensor("negpad", [128, NBLK_CORE], f32,
                          kind="ExternalInput").ap()
    id_d = nc.dram_tensor("ident", [128, 128], bf, kind="ExternalInput").ap()
    w1_d = nc.dram_tensor("w1", [D, D], bf, kind="ExternalInput").ap()
    w2_d = nc.dram_tensor("w2", [D, D], bf, kind="ExternalInput").ap()
    wc1_d = nc.dram_tensor("wc1", [D, D], bf, kind="ExternalInput").ap()
    wc2_d = nc.dram_tensor("wc2", [D, D], bf, kind="ExternalInput").ap()
    if has_msg_bias:
        bmsg_d = nc.dram_tensor("bmsg_bf", [1, D], bf,
                                kind="ExternalInput").ap()
    if has_comb_bias:
        bcomb_d = nc.dram_tensor("bcomb_bf", [1, D], bf,
                                 kind="ExternalInput").ap()
    if has_msg_bias or has_comb_bias:
        ones_d = nc.dram_tensor("ones_bf", [1, 512], bf,
                                kind="ExternalInput").ap()
    outT = nc.dram_tensor("outT", [D, NVC], bf, kind="ExternalOutput").ap()

    with tile.TileContext(nc) as tc:
        with (tc.tile_pool(name="const", bufs=1) as constp,
              tc.tile_pool(name="stage", bufs=3) as stagep,
              tc.tile_pool(name="work", bufs=3) as workp,
              tc.tile_pool(name="psum_m", bufs=4, space="PSUM") as psmp,
              tc.tile_pool(name="psum_a", bufs=2, space="PSUM") as psap,
              tc.tile_pool(name="psum_p", bufs=1, space="PSUM") as pspp,
              tc.tile_pool(name="psum_b", bufs=1, space="PSUM") as psbp):

            # Head ordering: the tile framework gates consumers on all
            # earlier-issued DMAs, so issue only what the p-pipeline needs
            # (vT, w1, negpad), then emit the group-0 p compute, then the
            # remaining consts and the g0 xj stream.
            vT_st0 = stagep.tile([128, GROUP * 128], bf, tag="vT")
            nc.sync.dma_start(vT_st0[:, :min(GROUP, NBLK_CORE) * 128],
                              vT_d[:, :min(GROUP, NBLK_CORE) * 128])
            w1_s = constp.tile([D, D], bf)
            nc.sync.dma_start(w1_s[:], w1_d[:])
            np_s = constp.tile([128, NBLK_CORE], f32)
            nc.sync.dma_start(np_s[:], np_d[:])
            id_s = constp.tile([128, 128], bf)
            w2_s = constp.tile([D, D], bf)
            wc1_s = constp.tile([D, D], bf)
            wc2_s = constp.tile([D, D], bf)
            xj_st0 = stagep.tile([128, max_grp_tiles * 128], f8, tag="xj")

            def _late_head_dmas():
                nc.sync.dma_start(id_s[:], id_d[:])
                nc.sync.dma_start(w2_s[:], w2_d[:])
                _nb0 = min(GROUP, NBLK_CORE)
                _o = 0
                for _b in range(_nb0):
                    _w = T_j[_b] * 128
                    if _b == 0:
                        _h = 4 * 128
                        nc.sync.dma_start(xj_st0[:, :_h], xj8_d[:, :_h])
                        nc.sync.dma_start(xj_st0[:, _h:_w],
                                          xj8_d[:, _h:_w])
                    else:
                        nc.sync.dma_start(xj_st0[:, _o:_o + _w],
                                          xj8_d[:, _o:_o + _w])
                    _o += _w
                nc.sync.dma_start(wc1_s[:], wc1_d[:])
                nc.sync.dma_start(wc2_s[:], wc2_d[:])
            if has_msg_bias:
                bmsg_s = constp.tile([1, D], bf)
                nc.sync.dma_start(bmsg_s[:], bmsg_d[:])
            if has_comb_bias:
                bcomb_s = constp.tile([1, D], bf)
                nc.sync.dma_start(bcomb_s[:], bcomb_d[:])
            if has_msg_bias or has_comb_bias:
                ones_s = constp.tile([1, 512], bf)
                nc.sync.dma_start(ones_s[:], ones_d[:])

            for _rep in range(repeat):
                eng_flip = 0
                state = {}

                def stageA(g, pre=None):
                    nb = min(GROUP, NBLK_CORE - g * GROUP)
                    Tg = T_j[g * GROUP:g * GROUP + nb]
                    nch = sum(Tg)
                    nsl = nch * 128
                    s0 = off_j[g * GROUP] * 128
                    nvw = nb * 128

                    if pre is not None:
                        xj_st, vT_st = pre
                    else:
                        xj_st = stagep.tile([128, max_grp_tiles * 128],
                                            f8, tag="xj")
                        nc.sync.dma_start(xj_st[:, :nsl],
                                          xj8_d[:, s0:s0 + nsl])
                        vT_st = stagep.tile([128, GROUP * 128], bf,
                                            tag="vT")
                        nc.sync.dma_start(
                            vT_st[:, :nvw],
                            vT_d[:, g * GROUP * 128:g * GROUP * 128
                                 + nvw])

                    pp = pspp.tile([128, 512], f32, tag="pp")
                    for b in range(nb):
                        nc.tensor.matmul(
                            pp[:, b * 128:(b + 1) * 128],
                            vT_st[:, b * 128:(b + 1) * 128], w1_s[:],
                            start=(b == 0), stop=not has_msg_bias,
                            skip_group_check=True)
                        if has_msg_bias:
                            nc.tensor.matmul(
                                pp[:, b * 128:(b + 1) * 128],
                                ones_s[:, :128], bmsg_s[:],
                                start=False, stop=True,
                                skip_group_check=True)
                    p_s = workp.tile([128, 512], bf, tag="ps")
                    h = (nvw // 256) * 128
                    if h:
                        nc.scalar.copy(p_s[:, :h], pp[:, :h])
                        nc.vector.tensor_copy(p_s[:, h:nvw], pp[:, h:nvw])
                    else:
                        nc.scalar.copy(p_s[:, :nvw], pp[:, :nvw])

                    m_corr = workp.tile([128, 512], bf, tag="mc")
                    for b in range(nb):
                        j = g * GROUP + b
                        nc.vector.tensor_scalar(
                            m_corr[:, b * 128:(b + 1) * 128],
                            p_s[:, b * 128:(b + 1) * 128],
                            0.0, np_s[:, j:j + 1], op0=amax, op1=amult)
                    if pre is not None:
                        _late_head_dmas()
                    state[g] = dict(nb=nb, Tg=Tg, nch=nch, nvw=nvw,
                                    s0=s0, xj_st=xj_st, vT_st=vT_st,
                                    p_s=p_s, m_corr=m_corr)

                def stageB(g, mid_hook=None, early_hook=None):
                    nonlocal eng_flip
                    st = state[g]
                    nb, Tg, nch = st["nb"], st["Tg"], st["nch"]
                    nvw = st["nvw"]
                    xj_st, p_s, m_corr = (st["xj_st"], st["p_s"],
                                          st["m_corr"])
                    tlist = []
                    for b in range(nb):
                        for t in range(Tg[b]):
                            tlist.append((b, t))
                    nbank = (nch + 3) // 4
                    pa = psap.tile([128, 512], f32, tag="pa")
                    mss = [None] * nbank

                    def emit_mm12(jb):
                        nonlocal eng_flip
                        w = min(4, nch - jb * 4)
                        pm = psmp.tile([128, 512], f32, tag="pm")
                        jj = 0
                        while jj < w:
                            b = tlist[jb * 4 + jj][0]
                            r = jj
                            while r < w and tlist[jb * 4 + r][0] == b:
                                r += 1
                            run = r - jj
                            rhs = p_s[:, b * 128:(b + 1) * 128]
                            if run > 1:
                                rhs = rhs.unsqueeze(1).broadcast_to(
                                    [128, run, 128])
                            nc.tensor.matmul(
                                pm[:, jj * 128:(jj + run) * 128],
                                id_s[:], rhs, start=(jj == 0),
                                stop=False, skip_group_check=True)
                            jj = r
                        for jj in range(w):
                            cc = jb * 4 + jj
                            nc.tensor.matmul(
                                pm[:, jj * 128:(jj + 1) * 128],
                                xj_st[:, cc * 128:cc * 128 + 128],
                                w2_s[:],
                                start=False, stop=True,
                                skip_group_check=True)
                        m_s = workp.tile([128, 512], bf, tag="m", bufs=4)
                        mss[jb] = m_s
                        if eng_flip % 2 == 0:
                            nc.vector.tensor_scalar_max(
                                m_s[:, :w * 128], pm[:, :w * 128], 0.0)
                        else:
                            nc.scalar.activation(m_s[:, :w * 128],
                                                 pm[:, :w * 128], relu_t)
                        eng_flip += 1

                    def emit_mm3(jb):
                        w = min(4, nch - jb * 4)
                        m_s = mss[jb]
                        for jj in range(w):
                            b, t = tlist[jb * 4 + jj]
                            if t == 0:
                                nc.tensor.matmul(
                                    pa[:, b * 128:(b + 1) * 128],
                                    m_corr[:, b * 128:(b + 1) * 128],
                                    id_s[:], start=(b == 0), stop=False,
                                    skip_group_check=True)
                            nc.tensor.matmul(
                                pa[:, b * 128:(b + 1) * 128],
                                m_s[:, jj * 128:(jj + 1) * 128],
                                id_s[:],
                                start=False, stop=(t == Tg[b] - 1),
                                skip_group_check=True)

                    LAG = 2
                    for jb in range(nbank):
                        emit_mm12(jb)
                        if jb >= LAG:
                            emit_mm3(jb - LAG)
                        if jb == nbank // 2 and mid_hook is not None:
                            mid_hook()
                        if jb == LAG - 1 and early_hook is not None:
                            early_hook()
                    for jb in range(max(0, nbank - LAG), nbank):
                        emit_mm3(jb)

                    ag_s = workp.tile([128, 512], bf, tag="ag")
                    h = (nvw // 256) * 128
                    if h:
                        nc.scalar.copy(ag_s[:, :h], pa[:, :h])
                        nc.vector.tensor_copy(ag_s[:, h:nvw], pa[:, h:nvw])
                    else:
                        nc.scalar.copy(ag_s[:, :nvw], pa[:, :nvw])
                    st["ag_s"] = ag_s

                def stageC(g):
                    st = state.pop(g)
                    nvw, vT_st, ag_s = st["nvw"], st["vT_st"], st["ag_s"]
                    ph = psbp.tile([128, 512], f32, tag="ph")
                    nc.tensor.matmul(ph[:, :nvw], wc1_s[:],
                                     vT_st[:, :nvw],
                                     start=True, stop=False)
                    nc.tensor.matmul(ph[:, :nvw], wc2_s[:],
                                     ag_s[:, :nvw],
                                     start=False, stop=not has_comb_bias,
                                     skip_group_check=True)
                    if has_comb_bias:
                        nc.tensor.matmul(ph[:, :nvw], bcomb_s[:],
                                         ones_s[:, :nvw],
                                         start=False, stop=True,
                                         skip_group_check=True)
                    o_s = workp.tile([128, 512], bf, tag="o")
                    nc.vector.scalar_tensor_tensor(
                        o_s[:, :nvw], ph[:, :nvw], 0.0, vT_st[:, :nvw],
                        op0=amax, op1=mybir.AluOpType.add)
                    nc.sync.dma_start(
                        outT[:, g * GROUP * 128:g * GROUP * 128 + nvw],
                        o_s[:, :nvw])

                stageA(0, pre=(xj_st0, vT_st0))
                for g in range(NGROUPS):
                    hookA = ((lambda gg=g: stageA(gg + 1))
                             if g + 1 < NGROUPS else None)
                    hookC = ((lambda gg=g: stageC(gg - 1))
                             if g >= 1 else None)
                    stageB(g, mid_hook=hookA, early_hook=hookC)
                stageC(NGROUPS - 1)

    split_multi_waits(nc)
    return nc


_RUN_KW = {}   # test harness can inject run_bass_kernel_spmd kwargs
_REPEAT = 1    # test harness can ask for a repeated body (timing)


def kernel(variables, factors, v_to_f, f_to_v, edge_attr,
           W_msg, b_msg, W_comb, b_comb):
    variables = np.asarray(variables, np.float32)
    factors = np.asarray(factors, np.float32)
    v_to_f = np.asarray(v_to_f, np.int32)
    f_to_v = np.asarray(f_to_v, np.int32)
    W_msg = np.asarray(W_msg, np.float32)
    b_msg = np.asarray(b_msg, np.float32)
    W_comb = np.asarray(W_comb, np.float32)
    b_comb = np.asarray(b_comb, np.float32)

    in_maps, vid_of, T_j, has_mb, has_cb = build_host_data(
        variables, factors, v_to_f, f_to_v, W_msg, b_msg, W_comb, b_comb)

    nc = build_nc(T_j, has_mb, has_cb, repeat=_REPEAT)
    res = run_bass_kernel_spmd(nc, in_maps, list(range(NC)), **_RUN_KW)

    out_full = np.zeros((NV, D), np.float32)
    for c in range(NC):
        vids = vid_of[c].reshape(-1)
        mask = vids >= 0
        outc = res.results[c]["outT"].T.astype(np.float32)
        out_full[vids[mask]] = outc[mask]
    kernel.last_results = res
    return out_full
